# revision 1
# baseline (speedup 1.0000x reference)
"""DP-means (2-iteration early-stop) Trainium2 kernel, data-parallel over 8 NeuronCores.

The reference DP-means run on this problem's input converges at iteration 1
(the freeze flag engages at iteration 2; final K=3, convergence margin ~16x),
so the 50-step scan collapses to exactly two effective iterations:
  it0: mu0 = mean(X); far0: ||x-mu0||^2 > lambda; clusters {near0, far0}
  it1: distances to {mean(near0), mean(far0)}; argmin; far1 -> new cluster 2;
       final mu rows 0..2 = segment means of it1 assignments, rows 3..63 = 0.
The host validates the convergence/branch conditions from device-computed
aggregates and falls back to a full numpy DP-means if any assumption fails.

Sharding: points split across 8 cores (data parallel). Heavy compute is PE
matmuls: distances contract over features (from a host-pretransposed XT),
masked/segment sums contract over points (natural X with one-hot stationary
weights). Three AllReduces carry the global aggregates between phases.

Layout conventions (per shard of ns points):
  natural point index i = t*128 + p     (tile t, partition p); "pm" tensors are
  [128, ns/128] with pm[p, t] = value of point t*128+p.
  XT row index r = b*128 + q  <-> feature f = q*8 + b (columns in natural
  point order), so [1, D] feature vectors reshape to [128, 8] "muT" tiles with
  contiguous DMAs, and XT 128x128 column slices serve directly as matmul
  stationary operands producing pm-layout [128, nt] outputs in PSUM.
"""
import sys

sys.path.insert(0, "/opt/trn_rl_repo")

import numpy as np

import concourse.bass as bass
import concourse.bacc as bacc
import concourse.tile as tile
from concourse import mybir
from concourse import bass_utils

F32 = mybir.dt.float32
BF16 = mybir.dt.bfloat16
Alu = mybir.AluOpType
Act = mybir.ActivationFunctionType

N_FULL, D, K_MAX = 65536, 1024, 64
NCORES = 8
LAMBDA = 1000.0
MAX_ITER = 50
OBJ_TOL = 1e-3
P = 128
NB = D // P  # 8 feature blocks


def build_kernel(ns: int, ncores: int, n_total: int, reps: int = 1, fake_coll: bool = False):
    nt = ns // P           # pm columns / natural tiles
    npc = ns // 512        # 512-point chunks for XT passes
    assert ns % 512 == 0

    nc = bacc.Bacc("TRN2", target_bir_lowering=False, debug=False,
                   num_devices=ncores)
    x = nc.dram_tensor("x", [ns, D], F32, kind="ExternalInput")
    xt = nc.dram_tensor("xt", [D, ns], BF16, kind="ExternalInput")
    xlo = nc.dram_tensor("xlo", [D, ns], BF16, kind="ExternalInput")
    o_colsum = nc.dram_tensor("o_colsum", [1, D], F32, kind="ExternalOutput")
    o_ar2 = nc.dram_tensor("o_ar2", [1, D + 3 * P], F32, kind="ExternalOutput")
    o_ar3 = nc.dram_tensor("o_ar3", [1, 3 * D + 5 * P], F32, kind="ExternalOutput")

    rg = [list(range(ncores))]

    with tile.TileContext(nc) as tc:
        with (
            tc.tile_pool(name="persist", bufs=1) as pp,
            tc.tile_pool(name="stream", bufs=6) as sp,
            tc.tile_pool(name="xts", bufs=4) as xp,
            tc.tile_pool(name="scratch", bufs=1) as scp,
            tc.tile_pool(name="psum", bufs=1, space="PSUM") as psp,
            tc.tile_pool(name="dram", bufs=1, space="DRAM") as dp,
        ):
            def _all_reduce(bi, bo):
                if fake_coll:
                    nc.sync.dma_start(bo[:], bi[:])
                else:
                    nc.gpsimd.collective_compute(
                        "AllReduce", Alu.add, replica_groups=rg,
                        ins=[bi.opt()], outs=[bo.opt()])

            def _body():
                ones = pp.tile([P, 1], F32)
                nc.gpsimd.memset(ones[:], 1.0)

                # persistent pm-layout tensors
                x2pm = pp.tile([P, nt], F32)
                xm0pm = pp.tile([P, nt], F32)
                d0pm = pp.tile([P, nt], F32)
                d1pm = pp.tile([P, nt], F32)
                far0 = pp.tile([P, nt], F32)
                A0 = pp.tile([P, nt], F32)
                A1 = pp.tile([P, nt], F32)
                far1 = pp.tile([P, nt], F32)
                A_all = pp.tile([P, 3 * nt], F32)
                tmpa = scp.tile([P, nt], F32, tag="tmpa")
                tmpb = scp.tile([P, nt], F32, tag="tmpb")


                muT0b = pp.tile([P, NB], BF16)
                mu0_2b = pp.tile([P, 2 * NB], BF16)
                muT12b = pp.tile([P, 2 * NB], BF16)
                mu4b = pp.tile([P, 4 * NB], BF16)
                muT0raw = pp.tile([P, NB], F32)   # global colsum, muT layout
                muT0 = pp.tile([P, NB], F32)
                SfT = pp.tile([P, NB], F32)
                c1T = pp.tile([P, NB], F32)
                mu0pT = pp.tile([P, NB], F32)
                muT12 = pp.tile([P, 2 * NB], F32)

                nfar_p = pp.tile([P, 1], F32)
                sfx2_p = pp.tile([P, 1], F32)
                sx2_p = pp.tile([P, 1], F32)
                cnt_parts = pp.tile([P, 3], F32)
                snfdmin = pp.tile([P, 1], F32)
                sfx2_1 = pp.tile([P, 1], F32)


                # broadcast scalar tiles [128,1]
                m02b = pp.tile([P, 1], F32)
                m02pb = pp.tile([P, 1], F32)
                mc1b = pp.tile([P, 1], F32)
                rnfb = pp.tile([P, 1], F32)
                rnnb = pp.tile([P, 1], F32)

                # DRAM bounces
                b1i = dp.tile([1, D], F32)
                b1o = dp.tile([1, D], F32)
                b2i = dp.tile([1, D + 3 * P], F32)
                b2o = dp.tile([1, D + 3 * P], F32)
                b3i = dp.tile([1, 3 * D + 5 * P], F32)
                b3o = dp.tile([1, 3 * D + 5 * P], F32)
                sdram = dp.tile([1, 8], F32)   # scalar staging: slots

                def bcast_from_dram(dst, slot):
                    """dst [128,1] <- sdram[0, slot] replicated."""
                    src = sdram[:, slot:slot + 1].rearrange("o u -> (o u)")
                    rep = bass.AP(src.tensor, src.offset, [[0, P]] + src.ap)
                    nc.sync.dma_start(dst[:], rep)

                def part_sum_to_slot(col, slot):
                    """sdram[0, slot] <- sum over partitions of col [128,1]."""
                    ps = psp.tile([1, 1], F32, tag="tiny")
                    nc.tensor.matmul(ps[:], col[:], ones[:], start=True, stop=True)
                    s11 = scp.tile([1, 1], F32, tag="s11")
                    nc.vector.tensor_copy(s11[:], ps[:])
                    nc.sync.dma_start(sdram[:, slot:slot + 1], s11[:])

                # ---------------- Stage 1: natural pass (colsum + x2) ----------
                cs_ps = psp.tile([1, D], F32, tag="bigacc")
                for t in range(nt):
                    xtile = sp.tile([P, D], F32, tag="xtile")
                    nc.sync.dma_start(xtile[:], x[t * P:(t + 1) * P, :])
                    sq = scp.tile([P, D], F32, tag="actscratch")
                    nc.scalar.activation(sq[:], xtile[:], Act.Square,
                                         accum_out=x2pm[:, t:t + 1])
                    for h in range(2):
                        nc.tensor.matmul(cs_ps[:, h * 512:(h + 1) * 512], ones[:],
                                         xtile[:, h * 512:(h + 1) * 512],
                                         start=(t == 0), stop=(t == nt - 1))
                cs_sb = scp.tile([1, D], F32, tag="row1")
                nc.vector.tensor_copy(cs_sb[:], cs_ps[:])
                nc.sync.dma_start(b1i[:], cs_sb[:])
                _all_reduce(b1i, b1o)
                nc.sync.dma_start(o_colsum[:], b1o[:])
                # muT0 = colsum/N  (muT layout: [q, b] <- colsum[q*8+b])
                nc.sync.dma_start(muT0raw[:],
                                  b1o[:].rearrange("o (q b) -> (o q) b", b=NB))
                nc.vector.tensor_scalar(muT0[:], muT0raw[:], 1.0 / n_total, None,
                                        op0=Alu.mult)
                nc.vector.tensor_copy(muT0b[:], muT0[:])
                mu0r = scp.tile([P, NB], F32, tag="mu0r")
                nc.vector.tensor_copy(mu0r[:], muT0b[:])
                nc.vector.tensor_tensor(mu0r[:], muT0[:], mu0r[:], op=Alu.subtract)
                mu0lob = scp.tile([P, NB], BF16, tag="mu0lob")
                nc.vector.tensor_copy(mu0lob[:], mu0r[:])
                mu0_2bv = mu0_2b[:].rearrange("p (b k) -> p b k", k=2)
                nc.vector.tensor_copy(mu0_2bv[:, :, 0],
                                      muT0b[:].rearrange("p (b u) -> p b u", u=1)[:, :, 0])
                nc.vector.tensor_copy(mu0_2bv[:, :, 1],
                                      mu0lob[:].rearrange("p (b u) -> p b u", u=1)[:, :, 0])
                # m02 = ||mu0||^2
                sq8 = scp.tile([P, NB], F32, tag="sq8")
                acc1 = scp.tile([P, 1], F32, tag="acc1")
                nc.scalar.activation(sq8[:], muT0[:], Act.Square, accum_out=acc1[:])
                part_sum_to_slot(acc1, 0)
                bcast_from_dram(m02b, 0)

                # ---------------- Stage 2: XT pass (xm0), pm-native psum -------
                xmacc2 = scp.tile([P, 2 * nt], F32, tag="xmacc2")
                xmacc1 = scp.tile([P, nt], F32, tag="xmacc1")
                for b in range(NB):
                    xthb = xp.tile([P, ns], BF16, tag="xthb")
                    nc.sync.dma_start(xthb[:], xt[b * P:(b + 1) * P, :])
                    xlob2 = xp.tile([P, ns], BF16, tag="xlob")
                    nc.sync.dma_start(xlob2[:], xlo[b * P:(b + 1) * P, :])
                    xm_ps = psp.tile([P, 2 * nt], F32, tag="mvpm", bufs=1)
                    xm_ps2 = psp.tile([P, nt], F32, tag="mvpmB", bufs=1)
                    for g in range(nt):
                        nc.tensor.matmul(xm_ps[:, 2 * g:2 * g + 2],
                                         xthb[:, g * P:(g + 1) * P],
                                         mu0_2b[:, 2 * b:2 * b + 2],
                                         start=True, stop=True)
                        nc.tensor.matmul(xm_ps2[:, g:g + 1],
                                         xlob2[:, g * P:(g + 1) * P],
                                         muT0b[:, b:b + 1],
                                         start=True, stop=True)
                    if b == 0:
                        nc.vector.tensor_copy(xmacc2[:], xm_ps[:])
                        nc.vector.tensor_copy(xmacc1[:], xm_ps2[:])
                    else:
                        nc.vector.tensor_tensor(xmacc2[:], xmacc2[:], xm_ps[:],
                                                op=Alu.add)
                        nc.vector.tensor_tensor(xmacc1[:], xmacc1[:], xm_ps2[:],
                                                op=Alu.add)
                xmacc2v = xmacc2[:].rearrange("p (g k) -> p g k", k=2)
                nc.vector.tensor_tensor(xm0pm[:], xmacc2v[:, :, 0],
                                        xmacc2v[:, :, 1], op=Alu.add)
                nc.vector.tensor_tensor(xm0pm[:], xm0pm[:], xmacc1[:], op=Alu.add)
                # far0 = (x2 - 2*xm0 + m02) > LAMBDA
                nc.vector.tensor_scalar(tmpa[:], xm0pm[:], -2.0, None, op0=Alu.mult)
                nc.vector.tensor_tensor(tmpb[:], tmpa[:], x2pm[:], op=Alu.add)
                nc.vector.tensor_scalar(tmpa[:], tmpb[:], m02b[:], None, op0=Alu.add)
                nc.vector.tensor_scalar(far0[:], tmpa[:], LAMBDA, None, op0=Alu.is_gt)
                nc.vector.tensor_reduce(nfar_p[:], far0[:], axis=mybir.AxisListType.X,
                                        op=Alu.add)
                nc.vector.tensor_tensor(tmpb[:], far0[:], x2pm[:], op=Alu.mult)
                nc.vector.tensor_reduce(sfx2_p[:], tmpb[:], axis=mybir.AxisListType.X,
                                        op=Alu.add)
                nc.vector.tensor_reduce(sx2_p[:], x2pm[:], axis=mybir.AxisListType.X,
                                        op=Alu.add)

                # ---------------- Stage 2b: natural pass (S_far) ---------------
                sf_ps = psp.tile([1, D], F32, tag="bigacc")
                for t in range(nt):
                    xtile = sp.tile([P, D], F32, tag="xtile")
                    nc.sync.dma_start(xtile[:], x[t * P:(t + 1) * P, :])
                    for h in range(2):
                        nc.tensor.matmul(sf_ps[:, h * 512:(h + 1) * 512],
                                         far0[:, t:t + 1],
                                         xtile[:, h * 512:(h + 1) * 512],
                                         start=(t == 0), stop=(t == nt - 1))
                sf_sb = scp.tile([1, D], F32, tag="row1")
                nc.vector.tensor_copy(sf_sb[:], sf_ps[:])
                nc.sync.dma_start(b2i[:, 0:D], sf_sb[:])
                nc.sync.dma_start(
                    b2i[:, D:D + P].rearrange("o (p u) -> (o p) u", u=1), nfar_p[:])
                nc.sync.dma_start(
                    b2i[:, D + P:D + 2 * P].rearrange("o (p u) -> (o p) u", u=1),
                    sfx2_p[:])
                nc.sync.dma_start(
                    b2i[:, D + 2 * P:D + 3 * P].rearrange("o (p u) -> (o p) u", u=1),
                    sx2_p[:])
                _all_reduce(b2i, b2o)
                nc.sync.dma_start(o_ar2[:], b2o[:])
                # global S_far (muT layout) and n_far
                nc.sync.dma_start(SfT[:],
                                  b2o[:, 0:D].rearrange("o (q b) -> (o q) b", b=NB))
                nfg = scp.tile([P, 1], F32, tag="nfg")
                nc.sync.dma_start(
                    nfg[:], b2o[:, D:D + P].rearrange("o (p u) -> (o p) u", u=1))
                # 1/n_far and 1/(N - n_far) broadcast
                ps = psp.tile([1, 1], F32, tag="tiny")
                nc.tensor.matmul(ps[:], nfg[:], ones[:], start=True, stop=True)
                nf11 = scp.tile([1, 1], F32, tag="s11")
                nc.vector.tensor_copy(nf11[:], ps[:])
                rnf11 = scp.tile([1, 1], F32, tag="s11b")
                nc.vector.reciprocal(rnf11[:], nf11[:])
                nc.sync.dma_start(sdram[:, 1:2], rnf11[:])
                bcast_from_dram(rnfb, 1)
                nn11 = scp.tile([1, 1], F32, tag="s11c")
                nc.vector.tensor_scalar(nn11[:], nf11[:], -1.0, float(n_total),
                                        op0=Alu.mult, op1=Alu.add)
                rnn11 = scp.tile([1, 1], F32, tag="s11d")
                nc.vector.reciprocal(rnn11[:], nn11[:])
                nc.sync.dma_start(sdram[:, 2:3], rnn11[:])
                bcast_from_dram(rnnb, 2)
                # c1 = S_far/n_far ; mu0' = (colsum - S_far)/(N - n_far)
                nc.vector.tensor_scalar(c1T[:], SfT[:], rnfb[:], None, op0=Alu.mult)
                t8 = scp.tile([P, NB], F32, tag="t8")
                nc.vector.tensor_tensor(t8[:], muT0raw[:], SfT[:], op=Alu.subtract)
                nc.vector.tensor_scalar(mu0pT[:], t8[:], rnnb[:], None, op0=Alu.mult)
                muT12v = muT12[:].rearrange("p (b k) -> p b k", k=2)
                nc.vector.tensor_copy(muT12v[:, :, 0], mu0pT[:])
                nc.vector.tensor_copy(muT12v[:, :, 1], c1T[:])
                nc.vector.tensor_copy(muT12b[:], muT12[:])
                # split-precision residual: mulo = bf16(muT12 - fp32(muT12b))
                mur = scp.tile([P, 2 * NB], F32, tag="mur")
                nc.vector.tensor_copy(mur[:], muT12b[:])
                nc.vector.tensor_tensor(mur[:], muT12[:], mur[:], op=Alu.subtract)
                mulob = scp.tile([P, 2 * NB], BF16, tag="mulob")
                nc.vector.tensor_copy(mulob[:], mur[:])
                mu4bv = mu4b[:].rearrange("p (b k) -> p b k", k=4)
                muT12bv2 = muT12b[:].rearrange("p (b k) -> p b k", k=2)
                mulobv = mulob[:].rearrange("p (b k) -> p b k", k=2)
                nc.vector.tensor_copy(mu4bv[:, :, 0:2], muT12bv2[:])
                nc.vector.tensor_copy(mu4bv[:, :, 2:4], mulobv[:])
                # ||mu0'||^2, ||c1||^2
                nc.scalar.activation(sq8[:], mu0pT[:], Act.Square, accum_out=acc1[:])
                part_sum_to_slot(acc1, 3)
                bcast_from_dram(m02pb, 3)
                nc.scalar.activation(sq8[:], c1T[:], Act.Square, accum_out=acc1[:])
                part_sum_to_slot(acc1, 4)
                bcast_from_dram(mc1b, 4)

                # ---------------- Stage 3: XT pass (dist to mu0', c1) ----------
                dacc4 = scp.tile([P, 4 * nt], F32, tag="dacc4")
                dacc2 = scp.tile([P, 2 * nt], F32, tag="dacc2")
                for b in range(NB):
                    xthb = xp.tile([P, ns], BF16, tag="xthb")
                    nc.sync.dma_start(xthb[:], xt[b * P:(b + 1) * P, :])
                    xlob = xp.tile([P, ns], BF16, tag="xlob")
                    nc.sync.dma_start(xlob[:], xlo[b * P:(b + 1) * P, :])
                    ps_a = psp.tile([P, 4 * nt], F32, tag="mvpm2a", bufs=2)
                    ps_b = psp.tile([P, 2 * nt], F32, tag="mvpm2b", bufs=1)
                    for g in range(nt):
                        nc.tensor.matmul(ps_a[:, 4 * g:4 * g + 4],
                                         xthb[:, g * P:(g + 1) * P],
                                         mu4b[:, 4 * b:4 * b + 4],
                                         start=True, stop=True)
                        nc.tensor.matmul(ps_b[:, 2 * g:2 * g + 2],
                                         xlob[:, g * P:(g + 1) * P],
                                         muT12b[:, 2 * b:2 * b + 2],
                                         start=True, stop=True)
                    if b == 0:
                        nc.vector.tensor_copy(dacc4[:], ps_a[:])
                        nc.vector.tensor_copy(dacc2[:], ps_b[:])
                    else:
                        nc.vector.tensor_tensor(dacc4[:], dacc4[:], ps_a[:],
                                                op=Alu.add)
                        nc.vector.tensor_tensor(dacc2[:], dacc2[:], ps_b[:],
                                                op=Alu.add)
                dacc4v = dacc4[:].rearrange("p (g k) -> p g k", k=4)
                dacc2v = dacc2[:].rearrange("p (g k) -> p g k", k=2)
                # d = x_hi*mu_hi + x_hi*mu_lo + x_lo*mu_hi
                nc.vector.tensor_tensor(d0pm[:], dacc4v[:, :, 0], dacc4v[:, :, 2],
                                        op=Alu.add)
                nc.vector.tensor_tensor(d0pm[:], d0pm[:], dacc2v[:, :, 0],
                                        op=Alu.add)
                nc.vector.tensor_tensor(d1pm[:], dacc4v[:, :, 1], dacc4v[:, :, 3],
                                        op=Alu.add)
                nc.vector.tensor_tensor(d1pm[:], d1pm[:], dacc2v[:, :, 1],
                                        op=Alu.add)
                # dist0/dist1; z; far1; one-hot columns
                dist0 = scp.tile([P, nt], F32, tag="dist0")
                dist1 = scp.tile([P, nt], F32, tag="dist1")
                nc.vector.tensor_scalar(tmpa[:], d0pm[:], -2.0, None, op0=Alu.mult)
                nc.vector.tensor_tensor(tmpb[:], tmpa[:], x2pm[:], op=Alu.add)
                nc.vector.tensor_scalar(dist0[:], tmpb[:], m02pb[:], None, op0=Alu.add)
                nc.vector.tensor_scalar(tmpa[:], d1pm[:], -2.0, None, op0=Alu.mult)
                nc.vector.tensor_tensor(tmpb[:], tmpa[:], x2pm[:], op=Alu.add)
                nc.vector.tensor_scalar(dist1[:], tmpb[:], mc1b[:], None, op0=Alu.add)
                z1 = scp.tile([P, nt], F32, tag="z1")
                dmin = scp.tile([P, nt], F32, tag="dmin")
                nc.vector.tensor_tensor(z1[:], dist1[:], dist0[:], op=Alu.is_lt)
                nc.vector.tensor_tensor(dmin[:], dist0[:], dist1[:], op=Alu.min)
                nc.vector.tensor_scalar(far1[:], dmin[:], LAMBDA, None, op0=Alu.is_gt)
                nf1 = scp.tile([P, nt], F32, tag="nf1")
                z1c = scp.tile([P, nt], F32, tag="z1c")
                nc.vector.tensor_scalar(nf1[:], far1[:], -1.0, 1.0,
                                        op0=Alu.mult, op1=Alu.add)
                nc.vector.tensor_scalar(z1c[:], z1[:], -1.0, 1.0,
                                        op0=Alu.mult, op1=Alu.add)
                nc.vector.tensor_tensor(A0[:], z1c[:], nf1[:], op=Alu.mult)
                nc.vector.tensor_tensor(A1[:], z1[:], nf1[:], op=Alu.mult)
                # counts + objective pieces
                nc.vector.tensor_reduce(cnt_parts[:, 0:1], A0[:],
                                        axis=mybir.AxisListType.X, op=Alu.add)
                nc.vector.tensor_reduce(cnt_parts[:, 1:2], A1[:],
                                        axis=mybir.AxisListType.X, op=Alu.add)
                nc.vector.tensor_reduce(cnt_parts[:, 2:3], far1[:],
                                        axis=mybir.AxisListType.X, op=Alu.add)
                nc.vector.tensor_tensor(tmpb[:], dmin[:], nf1[:], op=Alu.mult)
                nc.vector.tensor_reduce(snfdmin[:], tmpb[:],
                                        axis=mybir.AxisListType.X, op=Alu.add)
                nc.vector.tensor_tensor(tmpb[:], x2pm[:], far1[:], op=Alu.mult)
                nc.vector.tensor_reduce(sfx2_1[:], tmpb[:],
                                        axis=mybir.AxisListType.X, op=Alu.add)
                # interleave one-hot A for segment-sum lhsT
                A_allv = A_all[:].rearrange("p (t k) -> p t k", k=3)
                nc.vector.tensor_copy(A_allv[:, :, 0], A0[:])
                nc.vector.tensor_copy(A_allv[:, :, 1], A1[:])
                nc.vector.tensor_copy(A_allv[:, :, 2], far1[:])

                # ---------------- Stage 3b: natural pass (segment sums) --------
                s3_ps = psp.tile([3, D], F32, tag="bigacc")
                for t in range(nt):
                    xtile = sp.tile([P, D], F32, tag="xtile")
                    nc.sync.dma_start(xtile[:], x[t * P:(t + 1) * P, :])
                    for h in range(2):
                        nc.tensor.matmul(s3_ps[:, h * 512:(h + 1) * 512],
                                         A_all[:, 3 * t:3 * t + 3],
                                         xtile[:, h * 512:(h + 1) * 512],
                                         start=(t == 0), stop=(t == nt - 1))
                s3_sb = scp.tile([3, D], F32, tag="row3")
                nc.vector.tensor_copy(s3_sb[:], s3_ps[:])
                nc.sync.dma_start(
                    b3i[:, 0:3 * D].rearrange("o (r d) -> (o r) d", d=D), s3_sb[:])
                nc.sync.dma_start(
                    b3i[:, 3 * D:3 * D + 3 * P].rearrange("o (p k) -> (o p) k", k=3),
                    cnt_parts[:])
                nc.sync.dma_start(
                    b3i[:, 3 * D + 3 * P:3 * D + 4 * P]
                    .rearrange("o (p u) -> (o p) u", u=1), snfdmin[:])
                nc.sync.dma_start(
                    b3i[:, 3 * D + 4 * P:3 * D + 5 * P]
                    .rearrange("o (p u) -> (o p) u", u=1), sfx2_1[:])
                _all_reduce(b3i, b3o)
                nc.sync.dma_start(o_ar3[:], b3o[:])

            for _rep in range(reps):
                _body()

    nc.compile()
    return nc


import ml_dtypes

def make_xt(x_shard: np.ndarray) -> np.ndarray:
    """Row-permuted bf16 transpose: xt[b*128+q, i] = x[i, q*8+b] (cols natural)."""
    ns, d = x_shard.shape
    xtn = x_shard.T  # [d, ns]
    xtr = xtn.reshape(P, NB, ns).transpose(1, 0, 2).reshape(d, ns)
    return np.ascontiguousarray(xtr).astype(ml_dtypes.bfloat16)


def make_xlo(x_shard: np.ndarray) -> np.ndarray:
    """bf16 residual of the XT layout: xlo = bf16(xt_f32 - fp32(bf16(xt_f32)))."""
    ns, d = x_shard.shape
    xtn = x_shard.T
    xtr = np.ascontiguousarray(
        xtn.reshape(P, NB, ns).transpose(1, 0, 2).reshape(d, ns))
    hi = xtr.astype(ml_dtypes.bfloat16)
    lo = (xtr - hi.astype(np.float32)).astype(ml_dtypes.bfloat16)
    return lo


_NC_CACHE = {}


def _get_nc(ns, ncores, n_total):
    key = (ns, ncores, n_total)
    if key not in _NC_CACHE:
        _NC_CACHE[key] = build_kernel(ns, ncores, n_total)
    return _NC_CACHE[key]


def _dpmeans_numpy_fallback(X):
    """Faithful full DP-means in numpy (slow); emergency correctness path."""
    n, d = X.shape
    mu = np.zeros((K_MAX, d), np.float32)
    mu[0] = X.mean(axis=0)
    K = 1
    x2 = np.sum(X * X, axis=1)
    prev_obj = 0.0
    for it in range(MAX_ITER):
        m2 = np.sum(mu * mu, axis=1)
        dist = x2[:, None] - 2.0 * (X @ mu.T) + m2[None, :]
        dist[:, K:] = 1e30
        dmin = dist.min(axis=1)
        z = dist.argmin(axis=1)
        far = dmin > LAMBDA
        create = bool(far.any()) and K < K_MAX
        Kc = min(K, K_MAX - 1)
        nfar = float(far.sum())
        new_center = (far.astype(np.float32) @ X) / max(nfar, 1.0)
        if create:
            mu_c = mu.copy()
            mu_c[Kc] = new_center
        else:
            mu_c = mu
        if create:
            z = np.where(far, Kc, z)
            new_col = x2 - 2.0 * (X @ new_center) + float(new_center @ new_center)
            dvals = np.where(far, new_col, dmin)
        else:
            dvals = dmin
        K = K + int(create)
        counts = np.zeros(K_MAX, np.float32)
        np.add.at(counts, z, 1.0)
        sums = np.zeros((K_MAX, d), np.float32)
        np.add.at(sums, z, X)
        mu = np.where((counts > 0)[:, None],
                      sums / np.maximum(counts, 1.0)[:, None], mu_c)
        obj = float(dvals.sum()) + LAMBDA * K
        if it > 0 and abs(obj - prev_obj) < OBJ_TOL * obj:
            break
        prev_obj = obj
    return mu


def run_device(X: np.ndarray, ncores: int = NCORES):
    """Run the 2-iteration device pipeline; returns (mu, diag) or None if the
    early-stop assumptions don't hold (caller then falls back)."""
    n, d = X.shape
    assert d == D
    ns = n // ncores
    nc = _get_nc(ns, ncores, n)
    in_maps = []
    for c in range(ncores):
        xs = np.ascontiguousarray(X[c * ns:(c + 1) * ns])
        in_maps.append({"x": xs, "xt": make_xt(xs), "xlo": make_xlo(xs)})
    res = bass_utils.run_bass_kernel_spmd(
        nc, in_maps, core_ids=list(range(ncores)))
    r = res.results[0]
    colsum = r["o_colsum"][0].astype(np.float64)
    ar2 = r["o_ar2"][0].astype(np.float64)
    ar3 = r["o_ar3"][0].astype(np.float64)
    S_far = ar2[0:D]
    nfar0 = ar2[D:D + P].sum()
    sfx2_0 = ar2[D + P:D + 2 * P].sum()
    sx2 = ar2[D + 2 * P:D + 3 * P].sum()
    sums = ar3[0:3 * D].reshape(3, D)
    cnts = ar3[3 * D:3 * D + 3 * P].reshape(P, 3).sum(axis=0)
    snfdmin = ar3[3 * D + 3 * P:3 * D + 4 * P].sum()
    sfx2_1 = ar3[3 * D + 4 * P:3 * D + 5 * P].sum()

    nn0 = n - nfar0
    # branch guards for the hardcoded 2-iteration schedule
    if not (nfar0 > 0.5 and nn0 > 0.5 and cnts.min() > 0.5):
        return None
    # objectives (host, from aggregates)
    mu0 = colsum / n
    m02 = float(mu0 @ mu0)
    S_near = colsum - S_far
    c1 = S_far / nfar0
    mu0p = S_near / nn0
    sum_near_d0 = (sx2 - sfx2_0) - 2.0 * float(S_near @ mu0) + nn0 * m02
    sum_far_d0 = sfx2_0 - 2.0 * float(S_far @ c1) + nfar0 * float(c1 @ c1)
    obj0 = sum_near_d0 + sum_far_d0 + LAMBDA * 2.0
    nfar1 = cnts[2]
    sum_far_d1 = sfx2_1 - float(sums[2] @ sums[2]) / nfar1
    obj1 = snfdmin + sum_far_d1 + LAMBDA * 3.0
    converged = abs(obj1 - obj0) < OBJ_TOL * obj1
    margin = abs(obj1 - obj0) / (OBJ_TOL * obj1)
    diag = dict(nfar0=nfar0, nfar1=nfar1, counts=cnts, obj0=obj0, obj1=obj1,
                margin=margin)
    if not converged or margin > 0.5:
        # not converged at it1 (or numerically too close to call): the
        # 2-iteration schedule would be wrong -> decline
        return None
    mu = np.zeros((K_MAX, D), np.float32)
    mu[0:3] = (sums / cnts[:, None]).astype(np.float32)
    return mu, diag


def kernel(x: np.ndarray) -> np.ndarray:
    X = np.asarray(x[0], dtype=np.float32)
    out = run_device(X)
    if out is None:
        mu = _dpmeans_numpy_fallback(X)
    else:
        mu, _ = out
    return mu[None, :, :]


if __name__ == "__main__":
    nc = build_kernel(8192, 8, N_FULL)
    print("built ok")



# revision 21
# speedup vs baseline: 1.2696x; 1.2696x over previous
"""DP-means (2-iteration early-stop) Trainium2 kernel, data-parallel over 8 NeuronCores.

The reference DP-means run on this problem's input converges at iteration 1
(final K=3, convergence margin ~27x), so the 50-step scan collapses to two
effective iterations:
  it0: mu0 = mean(X); far0: ||x-mu0||^2 > lambda; clusters {near0, far0}
  it1: distances to {mean(near0), mean(far0)}; argmin; far1 -> new cluster 2;
       final mu rows 0..2 = segment means of it1 assignments, rows 3..63 = 0.
The host validates the convergence/branch conditions from device-computed
aggregates and falls back to a full numpy DP-means if any assumption fails.

Precision scheme (validated vs the oracle): x is split as fp16 hi + lo.
fp16's 11-bit mantissa makes hi-only point-sums (S_far, segment sums)
accurate enough; colsum accumulates hi+lo into one PSUM group; per-point
x2 = Square(hi+lo) via a DVE add + ScalarE accumulate; the it1 distance
matvec uses 3 terms (hi*mu_hi + hi*mu_lo + lo*mu_hi) with the lo stream
stored as fp8 e4m3 pre-scaled by 2^12 (descaled on chip) to dodge fp8
underflow. All matmuls run at 1 cycle/row (no fp32 4-cycle paths).

Layout: natural-layout fp16 X stays RESIDENT in SBUF (128KB/partition) so
masked point-sums read it with zero DMA; matvecs stream the transposed
(XT, row-permuted) copy as the PE stationary operand. pm tensors are
[128, ns/128] with pm[p, t] = value of point t*128+p; XT row r = b*128+q
<-> feature f = q*8+b so [1, D] vectors reshape to [128, 8] muT tiles.
"""
import sys

sys.path.insert(0, "/opt/trn_rl_repo")

import numpy as np

import concourse.bass as bass
import concourse.bacc as bacc
import concourse.tile as tile
from concourse import mybir
from concourse import bass_utils

F32 = mybir.dt.float32
F16 = mybir.dt.float16
F8 = mybir.dt.float8e4
Alu = mybir.AluOpType
Act = mybir.ActivationFunctionType

N_FULL, D, K_MAX = 65536, 1024, 64
NCORES = 8
LAMBDA = 1000.0
MAX_ITER = 50
OBJ_TOL = 1e-3
P = 128
NB = D // P          # 8 feature blocks
LO_SCALE = 4096.0    # 2^12 pre-scale on the fp8 lo stream


def build_kernel(ns: int, ncores: int, n_total: int, reps: int = 1, fake_coll: bool = False,
                 debug_taps: bool = False):
    nt = ns // P
    assert ns % 512 == 0

    nc = bacc.Bacc("TRN2", target_bir_lowering=False, debug=False,
                   num_devices=ncores)
    xh = nc.dram_tensor("xh", [ns, D], F16, kind="ExternalInput")
    xl = nc.dram_tensor("xl", [ns, D], F16, kind="ExternalInput")
    xth = nc.dram_tensor("xth", [D, ns], F16, kind="ExternalInput")
    xtl8 = nc.dram_tensor("xtl8", [D, ns], F8, kind="ExternalInput")
    o_colsum = nc.dram_tensor("o_colsum", [1, D], F32, kind="ExternalOutput")
    o_ar2 = nc.dram_tensor("o_ar2", [1, D + 3 * P], F32, kind="ExternalOutput")
    o_ar3 = nc.dram_tensor("o_ar3", [1, 3 * D + 5 * P], F32, kind="ExternalOutput")
    o_tap = None
    if debug_taps:
        # 8 pm-layout taps: x2, xm0, far0, d0raw, d1raw, z1, A0, A1
        o_tap = nc.dram_tensor("o_tap", [8 * P, ns // P], F32,
                               kind="ExternalOutput")

    rg = [list(range(ncores))]

    with tile.TileContext(nc) as tc:
        with (
            tc.tile_pool(name="persist", bufs=1) as pp,
            tc.tile_pool(name="stream", bufs=2) as sp,
            tc.tile_pool(name="xts", bufs=2) as xp,
            tc.tile_pool(name="scratch", bufs=1) as scp,
            tc.tile_pool(name="psum", bufs=1, space="PSUM") as psp,
            tc.tile_pool(name="dram", bufs=1, space="DRAM") as dp,
        ):
            def _all_reduce(bi, bo):
                if fake_coll:
                    nc.sync.dma_start(bo[:], bi[:])
                else:
                    nc.gpsimd.collective_compute(
                        "AllReduce", Alu.add, replica_groups=rg,
                        ins=[bi.opt()], outs=[bo.opt()])

            def _body():
                ones16 = pp.tile([P, 1], F16)
                nc.gpsimd.memset(ones16[:], 1.0)
                onesf = pp.tile([P, 1], F32)
                nc.gpsimd.memset(onesf[:], 1.0)

                # resident natural-layout fp16 X: slice t is [:, t*D:(t+1)*D]
                xhres = pp.tile([P, nt * D], F16)

                # persistent pm-layout tensors
                x2pm = pp.tile([P, nt], F32)
                far0 = pp.tile([P, nt], F32)
                far16 = pp.tile([P, nt], F16)
                d0pm = pp.tile([P, nt], F32)
                d1pm = pp.tile([P, nt], F32)
                A_all = pp.tile([P, 3 * nt], F16)
                tmpa = scp.tile([P, nt], F32, tag="tmpa")
                tmpb = scp.tile([P, nt], F32, tag="tmpb")

                muT0 = pp.tile([P, NB], F32)
                muT0raw = pp.tile([P, NB], F32)
                mu0f16 = pp.tile([P, NB], F16)
                SfT = pp.tile([P, NB], F32)
                c1T = pp.tile([P, NB], F32)
                mu0pT = pp.tile([P, NB], F32)
                mu4 = pp.tile([P, 4 * NB], F16)
                mu2_8 = pp.tile([P, 2 * NB], F8)

                nfar_p = pp.tile([P, 1], F32)
                sfx2_p = pp.tile([P, 1], F32)
                sx2_p = pp.tile([P, 1], F32)
                cnt_parts = pp.tile([P, 3], F32)
                snfdmin = pp.tile([P, 1], F32)
                sfx2_1 = pp.tile([P, 1], F32)

                # broadcast scalar tiles [128,1]
                m02b = pp.tile([P, 1], F32)
                m02pb = pp.tile([P, 1], F32)
                mc1b = pp.tile([P, 1], F32)
                rnfb = pp.tile([P, 1], F32)
                rnnb = pp.tile([P, 1], F32)

                # DRAM bounces
                b1i = dp.tile([1, D], F32)
                b1o = dp.tile([1, D], F32)
                b2i = dp.tile([1, D + 3 * P], F32)
                b2o = dp.tile([1, D + 3 * P], F32)
                b3i = dp.tile([1, 3 * D + 5 * P], F32)
                b3o = dp.tile([1, 3 * D + 5 * P], F32)
                sdram = dp.tile([1, 8], F32)

                def bcast_from_dram(dst, slot):
                    src = sdram[:, slot:slot + 1].rearrange("o u -> (o u)")
                    rep = bass.AP(src.tensor, src.offset, [[0, P]] + src.ap)
                    nc.sync.dma_start(dst[:], rep)

                def part_sum_to_slot(col, slot):
                    ps = psp.tile([1, 1], F32, tag="tiny")
                    nc.tensor.matmul(ps[:], col[:], onesf[:], start=True, stop=True)
                    s11 = scp.tile([1, 1], F32, tag="s11")
                    nc.vector.tensor_copy(s11[:], ps[:])
                    nc.sync.dma_start(sdram[:, slot:slot + 1], s11[:])

                def fp16_pair(src_f32, dst_hi, dst_lo):
                    """dst_hi = fp16(src); dst_lo = fp16(src - f32(dst_hi))."""
                    nc.vector.tensor_copy(dst_hi[:], src_f32[:])
                    up = scp.tile([P, NB], F32, tag="pair_up")
                    nc.vector.tensor_copy(up[:], dst_hi[:])
                    res = scp.tile([P, NB], F32, tag="pair_res")
                    nc.vector.tensor_tensor(res[:], src_f32[:], up[:],
                                            op=Alu.subtract)
                    nc.vector.tensor_copy(dst_lo[:], res[:])

                # ------------- Phase 1: resident load, colsum(hi+lo), x2 -------
                # PSUM note: matmul start=True clears the ENTIRE 2KB bank, so
                # each accumulating tile below owns whole banks and start is
                # set only on its very first matmul (per-element has_written
                # makes later first-writes overwrite rather than accumulate).
                cs_ps3 = psp.tile([3, D], F32, tag="bigacc")
                cs_ps = cs_ps3[0:1, :]
                for t in range(nt):
                    xslice = xhres[:, t * D:(t + 1) * D]
                    nc.sync.dma_start(xslice, xh[t * P:(t + 1) * P, :])
                    xlt = sp.tile([P, D], F16, tag="xlt")
                    nc.sync.dma_start(xlt[:], xl[t * P:(t + 1) * P, :])
                    s = sp.tile([P, D], F32, tag="hisum")
                    nc.vector.tensor_tensor(s[:], xslice, xlt[:], op=Alu.add)
                    sq = scp.tile([P, D], F16, tag="actscratch")
                    nc.scalar.activation(sq[:], s[:], Act.Square,
                                         accum_out=x2pm[:, t:t + 1])
                    for h in range(2):
                        reg = cs_ps[:, h * 512:(h + 1) * 512]
                        nc.tensor.matmul(reg, ones16[:],
                                         xslice[:, h * 512:(h + 1) * 512],
                                         start=(t == 0), stop=False)
                        nc.tensor.matmul(reg, ones16[:],
                                         xlt[:, h * 512:(h + 1) * 512],
                                         start=False, stop=(t == nt - 1))
                rowbig1 = scp.tile([3, D], F32, tag="rowbig")
                cs_sb = rowbig1[0:1, :]
                nc.vector.tensor_copy(cs_sb, cs_ps[:])
                nc.sync.dma_start(b1i[:], cs_sb)
                _all_reduce(b1i, b1o)
                nc.sync.dma_start(o_colsum[:], b1o[:])
                # muT0 = colsum/N  (muT layout: [q, b] <- colsum[q*8+b])
                nc.sync.dma_start(muT0raw[:],
                                  b1o[:].rearrange("o (q b) -> (o q) b", b=NB))
                nc.vector.tensor_scalar(muT0[:], muT0raw[:], 1.0 / n_total, None,
                                        op0=Alu.mult)
                nc.vector.tensor_copy(mu0f16[:], muT0[:])
                # m02 = ||mu0||^2
                sq8 = scp.tile([P, NB], F32, tag="sq8")
                acc1 = scp.tile([P, 1], F32, tag="acc1")
                nc.scalar.activation(sq8[:], muT0[:], Act.Square, accum_out=acc1[:])
                part_sum_to_slot(acc1, 0)
                bcast_from_dram(m02b, 0)

                # ------------- Phase 2: xm0 matvec, far0, S_far ---------------
                xm_psb = psp.tile([P, 512], F32, tag="xmps")
                xm_ps = xm_psb[:, 0:nt]
                hw_, ht_ = ns // 2, nt // 2
                for b in range(NB):
                    for half in range(2):
                        xthb = xp.tile([P, hw_], F16, tag="xthb")
                        nc.sync.dma_start(
                            xthb[:],
                            xth[b * P:(b + 1) * P, half * hw_:(half + 1) * hw_])
                        for g in range(ht_):
                            gg = half * ht_ + g
                            nc.tensor.matmul(
                                xm_ps[:, gg:gg + 1],
                                xthb[:, g * P:(g + 1) * P],
                                mu0f16[:, b:b + 1],
                                start=(b == 0 and half == 0 and g == 0),
                                stop=(b == NB - 1 and half == 1 and g == ht_ - 1))
                # far0 = (x2 - 2*xm0 + m02) > LAMBDA
                nc.vector.tensor_scalar(tmpa[:], xm_ps[:], -2.0, None, op0=Alu.mult)
                nc.vector.tensor_tensor(tmpb[:], tmpa[:], x2pm[:], op=Alu.add)
                nc.vector.tensor_scalar(tmpa[:], tmpb[:], m02b[:], None, op0=Alu.add)
                nc.vector.tensor_scalar(far0[:], tmpa[:], LAMBDA, None, op0=Alu.is_gt)
                nc.vector.tensor_copy(far16[:], far0[:])
                nc.vector.tensor_reduce(nfar_p[:], far0[:], axis=mybir.AxisListType.X,
                                        op=Alu.add)
                nc.vector.tensor_tensor(tmpb[:], far0[:], x2pm[:], op=Alu.mult)
                nc.vector.tensor_reduce(sfx2_p[:], tmpb[:], axis=mybir.AxisListType.X,
                                        op=Alu.add)
                nc.vector.tensor_reduce(sx2_p[:], x2pm[:], axis=mybir.AxisListType.X,
                                        op=Alu.add)
                if debug_taps:
                    nc.sync.dma_start(o_tap[0 * P:1 * P, :], x2pm[:])
                    xmcp = scp.tile([P, nt], F32, tag="xmcp")
                    nc.vector.tensor_copy(xmcp[:], xm_ps[:])
                    nc.sync.dma_start(o_tap[1 * P:2 * P, :], xmcp[:])
                    nc.sync.dma_start(o_tap[2 * P:3 * P, :], far0[:])
                # S_far = far0^T @ X  (hi-only, from resident)
                sf_ps = psp.tile([1, D], F32, tag="bigacc2")
                for t in range(nt):
                    for h in range(2):
                        nc.tensor.matmul(sf_ps[:, h * 512:(h + 1) * 512],
                                         far16[:, t:t + 1],
                                         xhres[:, t * D + h * 512:t * D + (h + 1) * 512],
                                         start=(t == 0), stop=(t == nt - 1))
                rowbig2 = scp.tile([3, D], F32, tag="rowbig")
                sf_sb = rowbig2[0:1, :]
                nc.vector.tensor_copy(sf_sb, sf_ps[:])
                nc.sync.dma_start(b2i[:, 0:D], sf_sb)
                nc.sync.dma_start(
                    b2i[:, D:D + P].rearrange("o (p u) -> (o p) u", u=1), nfar_p[:])
                nc.sync.dma_start(
                    b2i[:, D + P:D + 2 * P].rearrange("o (p u) -> (o p) u", u=1),
                    sfx2_p[:])
                nc.sync.dma_start(
                    b2i[:, D + 2 * P:D + 3 * P].rearrange("o (p u) -> (o p) u", u=1),
                    sx2_p[:])
                _all_reduce(b2i, b2o)
                nc.sync.dma_start(o_ar2[:], b2o[:])
                # global S_far (muT layout) and 1/n_far, 1/(N-n_far)
                nc.sync.dma_start(SfT[:],
                                  b2o[:, 0:D].rearrange("o (q b) -> (o q) b", b=NB))
                nfg = scp.tile([P, 1], F32, tag="nfg")
                nc.sync.dma_start(
                    nfg[:], b2o[:, D:D + P].rearrange("o (p u) -> (o p) u", u=1))
                ps = psp.tile([1, 1], F32, tag="tiny")
                nc.tensor.matmul(ps[:], nfg[:], onesf[:], start=True, stop=True)
                nf11 = scp.tile([1, 1], F32, tag="s11")
                nc.vector.tensor_copy(nf11[:], ps[:])
                rnf11 = scp.tile([1, 1], F32, tag="s11b")
                nc.vector.reciprocal(rnf11[:], nf11[:])
                nc.sync.dma_start(sdram[:, 1:2], rnf11[:])
                bcast_from_dram(rnfb, 1)
                nn11 = scp.tile([1, 1], F32, tag="s11c")
                nc.vector.tensor_scalar(nn11[:], nf11[:], -1.0, float(n_total),
                                        op0=Alu.mult, op1=Alu.add)
                rnn11 = scp.tile([1, 1], F32, tag="s11d")
                nc.vector.reciprocal(rnn11[:], nn11[:])
                nc.sync.dma_start(sdram[:, 2:3], rnn11[:])
                bcast_from_dram(rnnb, 2)
                # c1 = S_far/n_far ; mu0' = (colsum - S_far)/(N - n_far)
                nc.vector.tensor_scalar(c1T[:], SfT[:], rnfb[:], None, op0=Alu.mult)
                t8 = scp.tile([P, NB], F32, tag="t8")
                nc.vector.tensor_tensor(t8[:], muT0raw[:], SfT[:], op=Alu.subtract)
                nc.vector.tensor_scalar(mu0pT[:], t8[:], rnnb[:], None, op0=Alu.mult)
                # fp16 split pairs and fp8 hi copies, interleaved per fblock
                m0h = scp.tile([P, NB], F16, tag="m0h")
                m0l = scp.tile([P, NB], F16, tag="m0l")
                c1h = scp.tile([P, NB], F16, tag="c1h")
                c1l = scp.tile([P, NB], F16, tag="c1l")
                fp16_pair(mu0pT, m0h, m0l)
                fp16_pair(c1T, c1h, c1l)
                # build interleaves in f32 (full-word strided writes are safe;
                # sub-word strided writes clobber neighbor lanes), then cast
                # with one contiguous copy
                mu4f = scp.tile([P, 4 * NB], F32, tag="mu4f")
                mu4fv = mu4f[:].rearrange("p (b k) -> p b k", k=4)
                for j, src in enumerate((m0h, m0l, c1h, c1l)):
                    nc.vector.tensor_copy(
                        mu4fv[:, :, j],
                        src[:].rearrange("p (b u) -> p b u", u=1)[:, :, 0])
                nc.vector.tensor_copy(mu4[:], mu4f[:])
                mu2f = scp.tile([P, 2 * NB], F32, tag="mu2f")
                mu2fv = mu2f[:].rearrange("p (b k) -> p b k", k=2)
                for j, src in enumerate((m0h, c1h)):
                    nc.vector.tensor_copy(
                        mu2fv[:, :, j],
                        src[:].rearrange("p (b u) -> p b u", u=1)[:, :, 0])
                nc.vector.tensor_copy(mu2_8[:], mu2f[:])
                # ||mu0'||^2, ||c1||^2
                nc.scalar.activation(sq8[:], mu0pT[:], Act.Square, accum_out=acc1[:])
                part_sum_to_slot(acc1, 3)
                bcast_from_dram(m02pb, 3)
                nc.scalar.activation(sq8[:], c1T[:], Act.Square, accum_out=acc1[:])
                part_sum_to_slot(acc1, 4)
                bcast_from_dram(mc1b, 4)

                # ------------- Phase 3: it1 distances, z, segment sums --------
                d4_psb = psp.tile([P, 512], F32, tag="d4")
                d4_ps = d4_psb[:, 0:4 * nt]
                dlo_psb = psp.tile([P, 512], F32, tag="dlo")
                dlo_ps = dlo_psb[:, 0:2 * nt]
                for b in range(NB):
                    for half in range(2):
                        xthb = xp.tile([P, hw_], F16, tag="xthb")
                        nc.sync.dma_start(
                            xthb[:],
                            xth[b * P:(b + 1) * P, half * hw_:(half + 1) * hw_])
                        xt8b = xp.tile([P, hw_], F8, tag="xt8b")
                        nc.sync.dma_start(
                            xt8b[:],
                            xtl8[b * P:(b + 1) * P, half * hw_:(half + 1) * hw_])
                        first = (b == 0 and half == 0)
                        last = (b == NB - 1 and half == 1)
                        for g in range(ht_):
                            gg = half * ht_ + g
                            nc.tensor.matmul(
                                d4_ps[:, 4 * gg:4 * gg + 4],
                                xthb[:, g * P:(g + 1) * P],
                                mu4[:, 4 * b:4 * b + 4],
                                start=(first and g == 0),
                                stop=(last and g == ht_ - 1))
                            nc.tensor.matmul(
                                dlo_ps[:, 2 * gg:2 * gg + 2],
                                xt8b[:, g * P:(g + 1) * P],
                                mu2_8[:, 2 * b:2 * b + 2],
                                start=(first and g == 0),
                                stop=(last and g == ht_ - 1))
                d4v = d4_ps[:].rearrange("p (g k) -> p g k", k=4)
                dlov = dlo_ps[:].rearrange("p (g k) -> p g k", k=2)
                # d = x_hi*mu_hi + x_hi*mu_lo + 2^-12 * x_lo8*mu_hi
                # (only one PSUM operand per DVE op allowed)
                nc.vector.tensor_scalar(tmpa[:], dlov[:, :, 0], 1.0 / LO_SCALE, None,
                                        op0=Alu.mult)
                nc.vector.tensor_tensor(d0pm[:], d4v[:, :, 0], tmpa[:], op=Alu.add)
                nc.vector.tensor_tensor(d0pm[:], d4v[:, :, 1], d0pm[:], op=Alu.add)
                nc.vector.tensor_scalar(tmpa[:], dlov[:, :, 1], 1.0 / LO_SCALE, None,
                                        op0=Alu.mult)
                nc.vector.tensor_tensor(d1pm[:], d4v[:, :, 2], tmpa[:], op=Alu.add)
                nc.vector.tensor_tensor(d1pm[:], d4v[:, :, 3], d1pm[:], op=Alu.add)
                # dist0/dist1; z; far1; one-hot columns
                dist0 = scp.tile([P, nt], F32, tag="dist0")
                dist1 = scp.tile([P, nt], F32, tag="dist1")
                nc.vector.tensor_scalar(tmpa[:], d0pm[:], -2.0, None, op0=Alu.mult)
                nc.vector.tensor_tensor(tmpb[:], tmpa[:], x2pm[:], op=Alu.add)
                nc.vector.tensor_scalar(dist0[:], tmpb[:], m02pb[:], None, op0=Alu.add)
                nc.vector.tensor_scalar(tmpa[:], d1pm[:], -2.0, None, op0=Alu.mult)
                nc.vector.tensor_tensor(tmpb[:], tmpa[:], x2pm[:], op=Alu.add)
                nc.vector.tensor_scalar(dist1[:], tmpb[:], mc1b[:], None, op0=Alu.add)
                z1 = scp.tile([P, nt], F32, tag="z1")
                dmin = scp.tile([P, nt], F32, tag="dmin")
                nc.vector.tensor_tensor(z1[:], dist1[:], dist0[:], op=Alu.is_lt)
                nc.vector.tensor_tensor(dmin[:], dist0[:], dist1[:], op=Alu.min)
                far1 = scp.tile([P, nt], F32, tag="far1")
                nc.vector.tensor_scalar(far1[:], dmin[:], LAMBDA, None, op0=Alu.is_gt)
                nf1 = scp.tile([P, nt], F32, tag="nf1")
                z1c = scp.tile([P, nt], F32, tag="z1c")
                A0 = scp.tile([P, nt], F32, tag="A0")
                A1 = scp.tile([P, nt], F32, tag="A1")
                nc.vector.tensor_scalar(nf1[:], far1[:], -1.0, 1.0,
                                        op0=Alu.mult, op1=Alu.add)
                nc.vector.tensor_scalar(z1c[:], z1[:], -1.0, 1.0,
                                        op0=Alu.mult, op1=Alu.add)
                nc.vector.tensor_tensor(A0[:], z1c[:], nf1[:], op=Alu.mult)
                nc.vector.tensor_tensor(A1[:], z1[:], nf1[:], op=Alu.mult)
                if debug_taps:
                    nc.sync.dma_start(o_tap[3 * P:4 * P, :], d0pm[:])
                    nc.sync.dma_start(o_tap[4 * P:5 * P, :], d1pm[:])
                    nc.sync.dma_start(o_tap[5 * P:6 * P, :], z1[:])
                    nc.sync.dma_start(o_tap[6 * P:7 * P, :], A0[:])
                    nc.sync.dma_start(o_tap[7 * P:8 * P, :], A1[:])
                # counts + objective pieces
                nc.vector.tensor_reduce(cnt_parts[:, 0:1], A0[:],
                                        axis=mybir.AxisListType.X, op=Alu.add)
                nc.vector.tensor_reduce(cnt_parts[:, 1:2], A1[:],
                                        axis=mybir.AxisListType.X, op=Alu.add)
                nc.vector.tensor_reduce(cnt_parts[:, 2:3], far1[:],
                                        axis=mybir.AxisListType.X, op=Alu.add)
                nc.vector.tensor_tensor(tmpb[:], dmin[:], nf1[:], op=Alu.mult)
                nc.vector.tensor_reduce(snfdmin[:], tmpb[:],
                                        axis=mybir.AxisListType.X, op=Alu.add)
                nc.vector.tensor_tensor(tmpb[:], x2pm[:], far1[:], op=Alu.mult)
                nc.vector.tensor_reduce(sfx2_1[:], tmpb[:],
                                        axis=mybir.AxisListType.X, op=Alu.add)
                # interleave one-hot A in f32, then one contiguous cast to fp16
                A_allf = scp.tile([P, 3 * nt], F32, tag="A_allf")
                A_allv = A_allf[:].rearrange("p (t k) -> p t k", k=3)
                nc.vector.tensor_copy(A_allv[:, :, 0], A0[:])
                nc.vector.tensor_copy(A_allv[:, :, 1], A1[:])
                nc.vector.tensor_copy(A_allv[:, :, 2], far1[:])
                nc.vector.tensor_copy(A_all[:], A_allf[:])
                # segment sums from resident hi
                s3_ps = psp.tile([3, D], F32, tag="bigacc")
                for t in range(nt):
                    for h in range(2):
                        nc.tensor.matmul(s3_ps[:, h * 512:(h + 1) * 512],
                                         A_all[:, 3 * t:3 * t + 3],
                                         xhres[:, t * D + h * 512:t * D + (h + 1) * 512],
                                         start=(t == 0), stop=(t == nt - 1))
                s3_sb = scp.tile([3, D], F32, tag="rowbig")
                nc.vector.tensor_copy(s3_sb[:], s3_ps[:])
                nc.sync.dma_start(
                    b3i[:, 0:3 * D].rearrange("o (r d) -> (o r) d", d=D), s3_sb[:])
                nc.sync.dma_start(
                    b3i[:, 3 * D:3 * D + 3 * P].rearrange("o (p k) -> (o p) k", k=3),
                    cnt_parts[:])
                nc.sync.dma_start(
                    b3i[:, 3 * D + 3 * P:3 * D + 4 * P]
                    .rearrange("o (p u) -> (o p) u", u=1), snfdmin[:])
                nc.sync.dma_start(
                    b3i[:, 3 * D + 4 * P:3 * D + 5 * P]
                    .rearrange("o (p u) -> (o p) u", u=1), sfx2_1[:])
                _all_reduce(b3i, b3o)
                nc.sync.dma_start(o_ar3[:], b3o[:])

            for _rep in range(reps):
                _body()

    nc.compile()
    return nc


import ml_dtypes


def _xt_permute(a):
    """Row-permuted transpose: out[b*128+q, i] = a[i, q*8+b] (cols natural)."""
    ns, d = a.shape
    atn = a.T
    return np.ascontiguousarray(
        atn.reshape(P, NB, ns).transpose(1, 0, 2).reshape(d, ns))


def make_in_maps(X: np.ndarray, ncores: int):
    """Shard + encode the full (N, D) f32 X into per-core kernel inputs."""
    n = X.shape[0]
    ns = n // ncores
    in_maps = []
    for c in range(ncores):
        xs = np.ascontiguousarray(X[c * ns:(c + 1) * ns])
        hi = xs.astype(np.float16)
        lo32 = xs - hi.astype(np.float32)
        lo = lo32.astype(np.float16)
        xtl8 = (_xt_permute(lo32) * LO_SCALE).astype(ml_dtypes.float8_e4m3)
        in_maps.append({
            "xh": hi,
            "xl": lo,
            "xth": _xt_permute(xs).astype(np.float16),
            "xtl8": xtl8,
        })
    return in_maps


_NC_CACHE = {}


def _get_nc(ns, ncores, n_total):
    key = (ns, ncores, n_total)
    if key not in _NC_CACHE:
        _NC_CACHE[key] = build_kernel(ns, ncores, n_total)
    return _NC_CACHE[key]


def _dpmeans_numpy_fallback(X):
    """Faithful full DP-means in numpy (slow); emergency correctness path."""
    n, d = X.shape
    mu = np.zeros((K_MAX, d), np.float32)
    mu[0] = X.mean(axis=0)
    K = 1
    x2 = np.sum(X * X, axis=1)
    prev_obj = 0.0
    for it in range(MAX_ITER):
        m2 = np.sum(mu * mu, axis=1)
        dist = x2[:, None] - 2.0 * (X @ mu.T) + m2[None, :]
        dist[:, K:] = 1e30
        dmin = dist.min(axis=1)
        z = dist.argmin(axis=1)
        far = dmin > LAMBDA
        create = bool(far.any()) and K < K_MAX
        Kc = min(K, K_MAX - 1)
        nfar = float(far.sum())
        new_center = (far.astype(np.float32) @ X) / max(nfar, 1.0)
        if create:
            mu_c = mu.copy()
            mu_c[Kc] = new_center
        else:
            mu_c = mu
        if create:
            z = np.where(far, Kc, z)
            new_col = x2 - 2.0 * (X @ new_center) + float(new_center @ new_center)
            dvals = np.where(far, new_col, dmin)
        else:
            dvals = dmin
        K = K + int(create)
        counts = np.zeros(K_MAX, np.float32)
        np.add.at(counts, z, 1.0)
        sums = np.zeros((K_MAX, d), np.float32)
        np.add.at(sums, z, X)
        mu = np.where((counts > 0)[:, None],
                      sums / np.maximum(counts, 1.0)[:, None], mu_c)
        obj = float(dvals.sum()) + LAMBDA * K
        if it > 0 and abs(obj - prev_obj) < OBJ_TOL * obj:
            break
        prev_obj = obj
    return mu


def run_device(X: np.ndarray, ncores: int = NCORES):
    """Run the 2-iteration device pipeline; returns (mu, diag) or None if the
    early-stop assumptions don't hold (caller then falls back)."""
    n, d = X.shape
    assert d == D
    ns = n // ncores
    nc = _get_nc(ns, ncores, n)
    in_maps = make_in_maps(X, ncores)
    res = bass_utils.run_bass_kernel_spmd(
        nc, in_maps, core_ids=list(range(ncores)))
    r = res.results[0]
    colsum = r["o_colsum"][0].astype(np.float64)
    ar2 = r["o_ar2"][0].astype(np.float64)
    ar3 = r["o_ar3"][0].astype(np.float64)
    S_far = ar2[0:D]
    nfar0 = ar2[D:D + P].sum()
    sfx2_0 = ar2[D + P:D + 2 * P].sum()
    sx2 = ar2[D + 2 * P:D + 3 * P].sum()
    sums = ar3[0:3 * D].reshape(3, D)
    cnts = ar3[3 * D:3 * D + 3 * P].reshape(P, 3).sum(axis=0)
    snfdmin = ar3[3 * D + 3 * P:3 * D + 4 * P].sum()
    sfx2_1 = ar3[3 * D + 4 * P:3 * D + 5 * P].sum()

    nn0 = n - nfar0
    # branch guards for the hardcoded 2-iteration schedule
    if not (nfar0 > 0.5 and nn0 > 0.5 and cnts.min() > 0.5):
        return None
    # objectives (host, from aggregates)
    mu0 = colsum / n
    m02 = float(mu0 @ mu0)
    S_near = colsum - S_far
    c1 = S_far / nfar0
    mu0p = S_near / nn0
    sum_near_d0 = (sx2 - sfx2_0) - 2.0 * float(S_near @ mu0) + nn0 * m02
    sum_far_d0 = sfx2_0 - 2.0 * float(S_far @ c1) + nfar0 * float(c1 @ c1)
    obj0 = sum_near_d0 + sum_far_d0 + LAMBDA * 2.0
    nfar1 = cnts[2]
    sum_far_d1 = sfx2_1 - float(sums[2] @ sums[2]) / nfar1
    obj1 = snfdmin + sum_far_d1 + LAMBDA * 3.0
    converged = abs(obj1 - obj0) < OBJ_TOL * obj1
    margin = abs(obj1 - obj0) / (OBJ_TOL * obj1)
    diag = dict(nfar0=nfar0, nfar1=nfar1, counts=cnts, obj0=obj0, obj1=obj1,
                margin=margin)
    if not converged or margin > 0.5:
        return None
    mu = np.zeros((K_MAX, D), np.float32)
    mu[0:3] = (sums / cnts[:, None]).astype(np.float32)
    return mu, diag


def kernel(x: np.ndarray) -> np.ndarray:
    X = np.asarray(x[0], dtype=np.float32)
    out = run_device(X)
    if out is None:
        mu = _dpmeans_numpy_fallback(X)
    else:
        mu, _ = out
    return mu[None, :, :]


if __name__ == "__main__":
    nc = build_kernel(8192, 8, N_FULL)
    print("built ok")
